# revision 16
# baseline (speedup 1.0000x reference)
"""3-layer GCN on 8 Trainium2 NeuronCores (Bass/Tile).

Math: with A = D^-1/2 (Adj + I) D^-1/2 (PyG GCNConv norm, self-loops),
each layer is h' = leaky_relu((A h) W + b). Factor
A h = dinv * ((Adj+I)(dinv * h)), so aggregation is an unweighted
gather-sum over in-edges of the row-scaled table u = dinv*(h@W).

Sharding: nodes split 6250/core. Per layer each core computes its shard
of u (bf16, rows padded to 256B), chunked AllGathers replicate the full
table [50176, 128]bf16 to every core, then each core aggregates its own
targets via batched dma_gather over a host-built padded CSR:
 - targets degree-sorted per core -> near-uniform degree per 128-row block
 - each slot's rows (self + in-neighbors) balanced across two int16-index
   windows of the table: rows [0,32768) and [17408,50176)
 - blocks packed into groups of uniform (D_LO, D_HI) -> 2 dma_gather per
   group, in-place DVE fold-tree reduce, group-batched scale/bias/lrelu
 - AllGather is chunked per group (rank-major chunks; host row mapping
   accounts for it) so collective traffic overlaps compute
 - per-core pad slots (11 at the front, 11 at the back of the position
   space) have dinv forced to 0 on host, which keeps every table's pad
   rows exactly zero without any extra device work.
All index prep on host; all FLOPs on device.
"""
import os
import numpy as np
from contextlib import ExitStack

import concourse.bass as bass
import concourse.tile as tile
from concourse import bacc, mybir
from concourse.bass_utils import run_bass_kernel_spmd
from concourse.masks import make_identity

N = 50000
E = 800000
IN_F = 128
H = 96
EW = 128                 # padded table row width (bf16 -> 256B)
C_OUT = 21
CORES = 8
SH = N // CORES          # 6250 real nodes per core
NBLK = 49
SHP = NBLK * 128         # 6272 positions per core (22 pads: 11 front, 11 back)
NPAD_F = 11
TBL = CORES * SHP        # 50176 table rows
SLOPE = 0.01
WLO_END = 32768
WHI_BEG = TBL - 32768    # 17408
VOLCAP = 144             # max GRP*(D_LO+D_HI) wide-tile columns per group

F32 = mybir.dt.float32
BF16 = mybir.dt.bfloat16
I16 = mybir.dt.int16

LAST_RESULTS = None
MONO_AG = bool(int(os.environ.get("GCN_MONO_AG", "0")))
NOFOLD = bool(int(os.environ.get("GCN_NOFOLD", "0")))
BAR = bool(int(os.environ.get("GCN_BAR", "0")))
TRUNC = int(os.environ.get("GCN_TRUNC", "99"))


def _balance(nlo, nhi, nflex):
    take = min(nflex, max(0, (nhi + nflex - nlo + 1) // 2))
    return nlo + take, nhi + (nflex - take), take


def _host_prep(x, edge_index):
    src = np.asarray(edge_index[0], dtype=np.int64)
    tgt = np.asarray(edge_index[1], dtype=np.int64)
    deg = np.bincount(tgt, minlength=N).astype(np.float64) + 1.0
    dinv64 = 1.0 / np.sqrt(deg)
    core_of = tgt // SH

    orders, poss = [], []
    for c in range(CORES):
        indeg = deg[c * SH:(c + 1) * SH]
        order = np.argsort(-indeg, kind="stable")      # rank -> local node
        pos = np.empty(SH, dtype=np.int64)
        pos[order] = NPAD_F + np.arange(SH)            # local node -> position
        orders.append(order)
        poss.append(pos)

    # per-core edges bucketed by target position
    per_core = []
    for c in range(CORES):
        sel = core_of == c
        s_c = src[sel]
        t_c = tgt[sel]
        pt = poss[c][t_c - c * SH]
        o = np.argsort(pt, kind="stable")
        pt_s = pt[o]
        srcs = s_c[o]                                  # global src node ids
        cnt = np.bincount(pt_s, minlength=SHP)
        starts = np.zeros(SHP + 1, dtype=np.int64)
        np.cumsum(cnt, out=starts[1:])
        per_core.append((srcs, starts, cnt))

    # ---- pass A: provisional window counts with monolithic row mapping ----
    def count_pass(pos2row):
        slot_lo = np.zeros((CORES, SHP), dtype=np.int64)
        slot_hi = np.zeros((CORES, SHP), dtype=np.int64)
        for c in range(CORES):
            srcs, starts, cnt = per_core[c]
            src_core = srcs // SH
            src_pos = np.empty(len(srcs), dtype=np.int64)
            for cc in range(CORES):
                m = src_core == cc
                src_pos[m] = poss[cc][srcs[m] - cc * SH]
            r = pos2row[src_core, src_pos]
            lab = np.where(r < WHI_BEG, 0, np.where(r >= WLO_END, 1, 2))
            pt_of_edge = np.repeat(np.arange(SHP), cnt)
            nlo = np.zeros(SHP, dtype=np.int64)
            nhi = np.zeros(SHP, dtype=np.int64)
            nfx = np.zeros(SHP, dtype=np.int64)
            np.add.at(nlo, pt_of_edge[lab == 0], 1)
            np.add.at(nhi, pt_of_edge[lab == 1], 1)
            np.add.at(nfx, pt_of_edge[lab == 2], 1)
            # self rows (real positions only)
            pos_real = np.arange(NPAD_F, NPAD_F + SH)
            rs = pos2row[c, pos_real]
            s_lab = np.where(rs < WHI_BEG, 0, np.where(rs >= WLO_END, 1, 2))
            np.add.at(nlo, pos_real[s_lab == 0], 1)
            np.add.at(nhi, pos_real[s_lab == 1], 1)
            np.add.at(nfx, pos_real[s_lab == 2], 1)
            take = np.clip((nhi + nfx - nlo + 1) // 2, 0, nfx)
            slot_lo[c] = nlo + take
            slot_hi[c] = nhi + (nfx - take)
        return slot_lo, slot_hi

    mono = np.empty((CORES, SHP), dtype=np.int64)
    for c in range(CORES):
        mono[c] = c * SHP + np.arange(SHP)
    slot_lo, slot_hi = count_pass(mono)

    blk_lo = slot_lo.reshape(CORES, NBLK, 128).max(axis=(0, 2))
    blk_hi = slot_hi.reshape(CORES, NBLK, 128).max(axis=(0, 2))

    groups = []
    b = 0
    while b < NBLK:
        g = max(1, min(VOLCAP // max(int(blk_lo[b] + blk_hi[b]), 1), NBLK - b))
        while True:
            dlo = int(blk_lo[b:b + g].max())
            dhi = int(blk_hi[b:b + g].max())
            if g == 1 or g * (dlo + dhi) <= VOLCAP:
                break
            g -= 1
        groups.append([b, g, dlo, dhi])
        b += g

    # ---- chunk-major row mapping from groups ----
    if MONO_AG:
        pos2row = mono
    else:
        pos2row = np.empty((CORES, SHP), dtype=np.int64)
        for (b0, g, _, _) in groups:
            base = 128 * b0 * CORES
            for c in range(CORES):
                pr = np.arange(128 * b0, 128 * (b0 + g))
                pos2row[c, pr] = base + c * 128 * g + (pr - 128 * b0)

    LO_PAD = int(pos2row[0, 0])                  # core 0 front pad row (== 0)
    HI_PAD = int(pos2row[CORES - 1, SHP - 1]) - WHI_BEG  # last row - base
    assert LO_PAD < WHI_BEG and int(pos2row[CORES - 1, SHP - 1]) >= WLO_END

    # ---- pass B: real grids with the chunk-major mapping ----
    slot_lo, slot_hi = count_pass(pos2row)
    for grp in groups:
        b0, g = grp[0], grp[1]
        grp[2] = int(slot_lo[:, b0 * 128:(b0 + g) * 128].max())
        grp[3] = int(slot_hi[:, b0 * 128:(b0 + g) * 128].max())
    groups = [tuple(grp) for grp in groups]

    nbr16s = []
    total_cols = sum(8 * g * (dlo + dhi) for (_, g, dlo, dhi) in groups)
    for c in range(CORES):
        srcs, starts, cnt = per_core[c]
        src_core = srcs // SH
        src_pos = np.empty(len(srcs), dtype=np.int64)
        for cc in range(CORES):
            m = src_core == cc
            src_pos[m] = poss[cc][srcs[m] - cc * SH]
        rows_src = pos2row[src_core, src_pos]
        parts = []
        for (b0, g, dlo, dhi) in groups:
            lo_grid = np.full((g * 128, max(dlo, 1)), LO_PAD, dtype=np.int64)
            hi_grid = np.full((g * 128, max(dhi, 1)), HI_PAD, dtype=np.int64)
            for q in range(g * 128):
                p = b0 * 128 + q
                lst = rows_src[starts[p]:starts[p + 1]]
                if NPAD_F <= p < NPAD_F + SH:
                    lst = np.concatenate([lst, pos2row[c, p:p + 1]])
                lab = np.where(lst < WHI_BEG, 0, np.where(lst >= WLO_END, 1, 2))
                lo = list(lst[lab == 0])
                hi = list(lst[lab == 1])
                flex = list(lst[lab == 2])
                _, _, take = _balance(len(lo), len(hi), len(flex))
                lo += flex[:take]
                hi += flex[take:]
                assert len(lo) <= dlo and len(hi) <= dhi, (len(lo), dlo, len(hi), dhi)
                lo_grid[q, :len(lo)] = lo
                hi_grid[q, :len(hi)] = np.asarray(hi, dtype=np.int64) - WHI_BEG
            for grid, dd in ((lo_grid, dlo), (hi_grid, dhi)):
                if dd == 0:
                    continue
                a = grid[:, :dd].reshape(g, 128, dd).transpose(0, 2, 1).reshape(-1)
                parts.append(a)
        flat = np.concatenate(parts)
        assert flat.min() >= 0 and flat.max() < WLO_END
        m16 = flat.reshape(-1, 16).T.astype(np.int16)
        nbr16s.append(np.ascontiguousarray(np.tile(m16, (8, 1))))

    # ---- per-core xT, dinv arrays (zeros at pad slots) ----
    dinv32 = dinv64.astype(np.float32)
    xTs, dinv_blks, dinv_reps = [], [], []
    for c in range(CORES):
        xs = np.zeros((SHP, IN_F), dtype=np.float32)
        xs[NPAD_F:NPAD_F + SH] = np.asarray(
            x[c * SH:(c + 1) * SH], dtype=np.float32)[orders[c]]
        xTs.append(np.ascontiguousarray(xs.T))
        db = np.zeros(SHP, dtype=np.float32)
        db[NPAD_F:NPAD_F + SH] = dinv32[c * SH:(c + 1) * SH][orders[c]]
        dinv_blks.append(np.ascontiguousarray(db.reshape(NBLK, 128).T))
        rep = np.repeat(db.reshape(NBLK, 128, 1), H, axis=2)
        dinv_reps.append(np.ascontiguousarray(
            rep.transpose(1, 0, 2).reshape(128, NBLK * H)))

    return xTs, nbr16s, dinv_blks, dinv_reps, poss, (groups, total_cols)


def _fold_tree(nc, view, cols):
    cnt = cols
    while cnt > 1:
        h = cnt // 2
        nc.vector.tensor_tensor(
            out=view[:, :, 0:h, :], in0=view[:, :, 0:h, :],
            in1=view[:, :, cnt - h:cnt, :], op=mybir.AluOpType.add)
        cnt -= h


def _build_bass(groups, total_cols):
    nc = bacc.Bacc("TRN2", target_bir_lowering=False, debug=False,
                   num_devices=CORES)

    xT_in = nc.declare_dram_parameter("xT", [IN_F, SHP], F32, isOutput=False)
    nbr_in = nc.declare_dram_parameter("nbr16", [128, total_cols], I16,
                                       isOutput=False)
    dinv_in = nc.declare_dram_parameter("dinv_blk", [128, NBLK], F32, isOutput=False)
    dinvr_in = nc.declare_dram_parameter("dinv_rep", [128, NBLK * H], F32,
                                         isOutput=False)
    w_in = {}
    wspecs = [("W1", [IN_F, H]), ("W2", [H, H]), ("W3", [H, H]),
              ("Wl", [H, C_OUT]), ("B1", [128, H]), ("B2", [128, H]),
              ("B3", [128, H]), ("BL", [128, C_OUT])]
    for name, shp in wspecs:
        w_in[name] = nc.declare_dram_parameter(name, shp, F32, isOutput=False)
    out_dram = nc.declare_dram_parameter("out_s", [SHP, C_OUT], F32, isOutput=True)

    u_shard = [nc.dram_tensor(f"u_shard{l}", [SHP, EW], BF16) for l in range(3)]
    u_table = [nc.dram_tensor(f"u_table{l}", [TBL, EW], BF16) for l in range(3)]

    W_next = {0: "W2", 1: "W3"}
    B_of = {0: "B1", 1: "B2", 2: "B3"}

    def ag_issue(l, b0, g):
        if MONO_AG and b0 + g < NBLK:
            return
        if MONO_AG:
            b0, g = 0, NBLK
        nc.gpsimd.collective_compute(
            "AllGather", mybir.AluOpType.bypass,
            replica_groups=[list(range(CORES))],
            ins=[u_shard[l][b0 * 128:(b0 + g) * 128, :]],
            outs=[u_table[l][b0 * 128 * CORES:(b0 + g) * 128 * CORES, :]],
        )

    with tile.TileContext(nc) as tc, ExitStack() as ctx:
        const = ctx.enter_context(tc.tile_pool(name="const", bufs=1))
        widep = ctx.enter_context(tc.tile_pool(name="widep", bufs=2))
        work = ctx.enter_context(tc.tile_pool(name="work", bufs=3))
        outp = ctx.enter_context(tc.tile_pool(name="outp", bufs=3))
        psum = ctx.enter_context(tc.tile_pool(name="psum", bufs=2, space="PSUM"))

        xT = const.tile([IN_F, SHP], F32)
        nc.sync.dma_start(xT[:], xT_in[:])
        nbr = const.tile([128, total_cols], I16)
        nc.sync.dma_start(nbr[:], nbr_in[:])
        dinv = const.tile([128, NBLK], F32)
        nc.sync.dma_start(dinv[:], dinv_in[:])
        dinvr = const.tile([128, NBLK * H], F32)
        nc.sync.dma_start(dinvr[:], dinvr_in[:])
        wt = {}
        for name, shp in wspecs:
            t = const.tile(shp, F32, tag=name)
            nc.sync.dma_start(t[:], w_in[name][:])
            wt[name] = t
        ident = const.tile([128, 128], F32)
        make_identity(nc, ident[:])

        # ---- layer-1 table: u1 = dinv * (x @ W1) ----
        for (b0, g, dlo, dhi) in groups:
            for bg in range(g):
                b = b0 + bg
                vP = psum.tile([128, H], F32, tag="vP")
                nc.tensor.matmul(vP[:], lhsT=xT[:, b * 128:(b + 1) * 128],
                                 rhs=wt["W1"][:], start=True, stop=True)
                ub = work.tile([128, EW], BF16, tag="ub")
                nc.vector.tensor_scalar(ub[:, :H], vP[:], dinv[:, b:b + 1], None,
                                        op0=mybir.AluOpType.mult)
                nc.vector.memset(ub[:, H:], 0.0)
                nc.sync.dma_start(u_shard[0][b * 128:(b + 1) * 128, :], ub[:])
            if TRUNC >= 2:
                ag_issue(0, b0, g)

        # ---- three aggregation layers ----
        nlayers = 0 if TRUNC < 3 else (1 if TRUNC < 6 else (2 if TRUNC < 7 else 3))
        for l in range(nlayers):
            col_off = 0
            for (b0, g, dlo, dhi) in groups:
                wl = widep.tile([128, g * dlo, EW], BF16, tag="wl", name="wl") if dlo else None
                wh = widep.tile([128, g * dhi, EW], BF16, tag="wh", name="wh") if dhi else None
                GCAP = 8  # wide columns per dma_gather (1024-idx HW limit)
                for wt_, cols, win in ((wl, g * dlo, u_table[l][0:WLO_END, :]),
                                       (wh, g * dhi, u_table[l][WHI_BEG:TBL, :])):
                    k0 = 0
                    while k0 < cols:
                        kn = min(GCAP, cols - k0)
                        nc.gpsimd.dma_gather(
                            out_ap=wt_[:, k0:k0 + kn, :], in_ap=win,
                            idxs_ap=nbr[:, col_off:col_off + 8 * kn],
                            num_idxs=128 * kn, num_idxs_reg=128 * kn,
                            elem_size=EW)
                        col_off += 8 * kn
                        k0 += kn
                if TRUNC == 3:
                    continue
                if BAR:
                    tc.strict_bb_all_engine_barrier()
                vl = wl[:].rearrange("p (g j) e -> p g j e", g=g) if dlo else None
                vh = wh[:].rearrange("p (g j) e -> p g j e", g=g) if dhi else None
                sg = work.tile([128, g, H], F32, tag="sg")
                if NOFOLD:
                    s2 = work.tile([128, g, H], F32, tag="s2")
                    if dlo:
                        nc.vector.tensor_reduce(
                            sg[:], vl[:, :, :, :H].rearrange("p g j e -> p g e j"),
                            axis=mybir.AxisListType.X, op=mybir.AluOpType.add)
                    if dhi:
                        nc.vector.tensor_reduce(
                            s2[:], vh[:, :, :, :H].rearrange("p g j e -> p g e j"),
                            axis=mybir.AxisListType.X, op=mybir.AluOpType.add)
                    if dlo and dhi:
                        nc.vector.tensor_tensor(out=sg[:], in0=sg[:], in1=s2[:],
                                                op=mybir.AluOpType.add)
                    elif dhi:
                        nc.vector.tensor_copy(sg[:], s2[:])
                else:
                    if dlo:
                        _fold_tree(nc, vl, dlo)
                    if dhi:
                        _fold_tree(nc, vh, dhi)
                    if dlo and dhi:
                        nc.vector.tensor_tensor(out=sg[:], in0=vl[:, :, 0, :H],
                                                in1=vh[:, :, 0, :H],
                                                op=mybir.AluOpType.add)
                    elif dlo:
                        nc.vector.tensor_copy(sg[:], vl[:, :, 0, :H])
                    else:
                        nc.vector.tensor_copy(sg[:], vh[:, :, 0, :H])
                if TRUNC == 4:
                    continue
                dslice = dinvr[:, b0 * H:(b0 + g) * H].rearrange(
                    "p (g h) -> p g h", g=g)
                t1 = work.tile([128, g, H], F32, tag="t1")
                nc.vector.tensor_tensor(out=t1[:], in0=sg[:], in1=dslice,
                                        op=mybir.AluOpType.mult)
                bb = wt[B_of[l]][:].rearrange("p (o h) -> p o h", o=1)
                t2 = work.tile([128, g, H], F32, tag="t2")
                nc.vector.tensor_tensor(out=t2[:], in0=t1[:],
                                        in1=bb.to_broadcast([128, g, H]),
                                        op=mybir.AluOpType.add)
                t3 = work.tile([128, g, H], F32, tag="t3")
                nc.vector.tensor_scalar(t3[:], t2[:], SLOPE, None,
                                        op0=mybir.AluOpType.mult)
                hg = work.tile([128, g, H], F32, tag="hg")
                nc.vector.tensor_tensor(out=hg[:], in0=t2[:], in1=t3[:],
                                        op=mybir.AluOpType.max)
                if l < 2:
                    hsg = work.tile([128, g, H], F32, tag="hsg")
                    nc.vector.tensor_tensor(out=hsg[:], in0=hg[:], in1=dslice,
                                            op=mybir.AluOpType.mult)
                    for bg in range(g):
                        b = b0 + bg
                        trP = psum.tile([H, 128], F32, tag="trP")
                        nc.tensor.transpose(trP[:], hsg[:, bg, :], ident[:])
                        hsT = work.tile([H, 128], F32, tag="hsT")
                        nc.scalar.copy(hsT[:], trP[:])
                        vP = psum.tile([128, H], F32, tag="vP")
                        nc.tensor.matmul(vP[:], lhsT=hsT[:], rhs=wt[W_next[l]][:],
                                         start=True, stop=True)
                        ub = work.tile([128, EW], BF16, tag="ub")
                        nc.vector.tensor_copy(ub[:, :H], vP[:])
                        nc.vector.memset(ub[:, H:], 0.0)
                        nc.sync.dma_start(
                            u_shard[l + 1][b * 128:(b + 1) * 128, :], ub[:])
                    ag_issue(l + 1, b0, g)
                else:
                    for bg in range(g):
                        b = b0 + bg
                        trP = psum.tile([H, 128], F32, tag="trP")
                        nc.tensor.transpose(trP[:], hg[:, bg, :], ident[:])
                        hT = work.tile([H, 128], F32, tag="hsT")
                        nc.scalar.copy(hT[:], trP[:])
                        oP = psum.tile([128, C_OUT], F32, tag="oP")
                        nc.tensor.matmul(oP[:], lhsT=hT[:], rhs=wt["Wl"][:],
                                         start=True, stop=True)
                        o = outp.tile([128, C_OUT], F32, tag="o")
                        nc.vector.tensor_tensor(o[:], oP[:], wt["BL"][:],
                                                op=mybir.AluOpType.add)
                        nc.sync.dma_start(out_dram[b * 128:(b + 1) * 128, :], o[:])
    nc.compile()
    return nc


def _ensure_ntff_hook():
    import sys as _sys
    import types
    try:
        import antenv.axon_hooks  # noqa: F401
        return
    except ImportError:
        pass
    mod = types.ModuleType("antenv.axon_hooks")
    _h = [None]
    mod.set_axon_ntff_profile_hook = lambda hook: _h.__setitem__(0, hook)
    mod.get_axon_ntff_profile_hook = lambda: _h[0]
    _sys.modules["antenv.axon_hooks"] = mod
    try:
        from trn_agent_boot.trn_boot import _ntff_profile_via_ctypes
        hook = _ntff_profile_via_ctypes("/opt/axon/libaxon_pjrt.so")
        if hook is not None:
            mod.set_axon_ntff_profile_hook(hook)
    except Exception:
        pass


def kernel(x, edge_index, W1, b1, W2, b2, W3, b3, Wl, bl):
    global LAST_RESULTS
    x = np.asarray(x, dtype=np.float32)
    xTs, nbr16s, dinv_blks, dinv_reps, poss, meta = _host_prep(x, edge_index)
    groups, total_cols = meta

    nc = _build_bass(groups, total_cols)

    shared = {
        "W1": np.asarray(W1, np.float32), "W2": np.asarray(W2, np.float32),
        "W3": np.asarray(W3, np.float32), "Wl": np.asarray(Wl, np.float32),
        "B1": np.tile(np.asarray(b1, np.float32), (128, 1)),
        "B2": np.tile(np.asarray(b2, np.float32), (128, 1)),
        "B3": np.tile(np.asarray(b3, np.float32), (128, 1)),
        "BL": np.tile(np.asarray(bl, np.float32), (128, 1)),
    }
    in_maps = []
    for c in range(CORES):
        m = dict(shared)
        m["xT"] = xTs[c]
        m["nbr16"] = nbr16s[c]
        m["dinv_blk"] = dinv_blks[c]
        m["dinv_rep"] = dinv_reps[c]
        in_maps.append(m)

    trace = bool(int(os.environ.get("GCN_TRACE", "0")))
    if trace:
        _ensure_ntff_hook()
    res = run_bass_kernel_spmd(nc, in_maps, list(range(CORES)), trace=trace)
    LAST_RESULTS = res

    out = np.empty((N, C_OUT), dtype=np.float32)
    for c in range(CORES):
        shard = res.results[c]["out_s"]
        out[c * SH:(c + 1) * SH] = shard[poss[c]]
    return out


# revision 17
# speedup vs baseline: 1.3097x; 1.3097x over previous
"""3-layer GCN on 8 Trainium2 NeuronCores (Bass/Tile).

Math: with A = D^-1/2 (Adj + I) D^-1/2 (PyG GCNConv norm, self-loops),
each layer is h' = leaky_relu(A h W + b) = leaky_relu((A h) W + b).
Factor A h = dinv * ((Adj+I)(dinv * h)), so aggregation is an unweighted
gather-sum over in-edges of the row-scaled feature table u = dinv*(h@W).

Sharding: nodes are split 6250/core (8 cores). Per layer each core
computes its shard of the table u, an AllGather replicates the full
table [50176, 96] to every core's DRAM, then each core aggregates its
own targets with indirect-DMA row gathers over a host-built padded CSR
(targets degree-sorted per core so each 128-row block has near-uniform
degree). All index prep runs on host; all FLOPs on device.
"""
import os
import numpy as np
from contextlib import ExitStack

import concourse.bass as bass
import concourse.tile as tile
from concourse import bacc, mybir
from concourse.bass_utils import run_bass_kernel_spmd
from concourse.masks import make_identity

N = 50000
E = 800000
IN_F = 128
H = 96
C_OUT = 21
CORES = 8
SH = N // CORES          # 6250 real nodes per core
NBLK = 49                # ceil(6250/128)
SHP = NBLK * 128         # 6272 padded rows per shard
TBL = CORES * SHP        # 50176 table rows
SLOPE = 0.01

F32 = mybir.dt.float32
I32 = mybir.dt.int32

# stash for test.py introspection (exec time etc.)
LAST_RESULTS = None


def _host_prep(x, edge_index):
    """Build permutations, padded CSR (uniform per-block degree across cores),
    per-core transposed x, dinv blocks. Returns per-core input arrays and
    unpermute info."""
    src = np.asarray(edge_index[0], dtype=np.int64)
    tgt = np.asarray(edge_index[1], dtype=np.int64)
    deg = np.bincount(tgt, minlength=N).astype(np.float64) + 1.0
    dinv = (1.0 / np.sqrt(deg)).astype(np.float32)

    core_of = tgt // SH

    orders = []      # per core: position -> local node id
    poss = []        # per core: local node id -> position
    for c in range(CORES):
        indeg = deg[c * SH:(c + 1) * SH]
        order = np.argsort(-indeg, kind="stable")
        pos = np.empty(SH, dtype=np.int64)
        pos[order] = np.arange(SH)
        orders.append(order)
        poss.append(pos)

    # global node -> table row
    r_of = np.empty(N, dtype=np.int64)
    for c in range(CORES):
        r_of[c * SH:(c + 1) * SH] = c * SHP + poss[c]

    # per-core neighbor grids [SHP, Dmax_c] and lengths
    lens = np.zeros((CORES, SHP), dtype=np.int64)
    grids = []
    for c in range(CORES):
        sel = core_of == c
        s_c = src[sel]
        t_c = tgt[sel]
        pt = poss[c][t_c - c * SH]          # target position within shard
        o = np.argsort(pt, kind="stable")
        pt_s = pt[o]
        rows_src = r_of[s_c[o]].astype(np.int32)
        cnt = np.bincount(pt_s, minlength=SHP)
        lens[c] = cnt
        starts = np.zeros(SHP + 1, dtype=np.int64)
        np.cumsum(cnt, out=starts[1:])
        col = np.arange(len(pt_s)) - starts[pt_s]
        dmax = int(cnt.max()) if len(pt_s) else 0
        pad_row = np.int32(c * SHP + SH)    # a zeroed pad row of own shard
        grid = np.full((SHP, max(dmax, 1)), pad_row, dtype=np.int32)
        grid[pt_s, col] = rows_src
        grids.append(grid)

    # uniform per-block degree across cores
    D = lens.reshape(CORES, NBLK, 128).max(axis=(0, 2)).astype(np.int64)
    offs = np.zeros(NBLK + 1, dtype=np.int64)
    np.cumsum(D, out=offs[1:])
    sumd = int(offs[-1])

    nbrs = []
    for c in range(CORES):
        pad_row = np.int32(c * SHP + SH)
        nbr = np.full((128, sumd), pad_row, dtype=np.int32)
        g = grids[c]
        for b in range(NBLK):
            d = int(D[b])
            if d == 0:
                continue
            blk = g[b * 128:(b + 1) * 128, :min(d, g.shape[1])]
            nbr[:, offs[b]:offs[b] + blk.shape[1]] = blk
        nbrs.append(nbr)

    xTs, dinv_blks = [], []
    for c in range(CORES):
        xs = np.zeros((SHP, IN_F), dtype=np.float32)
        xs[:SH] = np.asarray(x[c * SH:(c + 1) * SH], dtype=np.float32)[orders[c]]
        xTs.append(np.ascontiguousarray(xs.T))
        db = np.ones(SHP, dtype=np.float32)
        db[:SH] = dinv[c * SH:(c + 1) * SH][orders[c]]
        dinv_blks.append(np.ascontiguousarray(db.reshape(NBLK, 128).T))
    return xTs, nbrs, dinv_blks, [int(d) for d in D], [int(o) for o in offs], poss


def _build_bass(D, offs, sumd):
    nc = bacc.Bacc("TRN2", target_bir_lowering=False, debug=False,
                   num_devices=CORES)

    xT_in = nc.declare_dram_parameter("xT", [IN_F, SHP], F32, isOutput=False)
    nbr_in = nc.declare_dram_parameter("nbr", [128, max(sumd, 1)], I32, isOutput=False)
    dinv_in = nc.declare_dram_parameter("dinv_blk", [128, NBLK], F32, isOutput=False)
    w_in = {}
    for name, shp in [("W1", [IN_F, H]), ("W2", [H, H]), ("W3", [H, H]),
                      ("Wl", [H, C_OUT]), ("B1", [128, H]), ("B2", [128, H]),
                      ("B3", [128, H]), ("BL", [128, C_OUT])]:
        w_in[name] = nc.declare_dram_parameter(name, shp, F32, isOutput=False)
    out_dram = nc.declare_dram_parameter("out_s", [SHP, C_OUT], F32, isOutput=True)

    u_shard = [nc.dram_tensor(f"u_shard{l}", [SHP, H], F32) for l in range(3)]
    u_table = [nc.dram_tensor(f"u_table{l}", [TBL, H], F32) for l in range(3)]

    W_next = {0: "W2", 1: "W3"}
    B_of = {0: "B1", 1: "B2", 2: "B3"}

    with tile.TileContext(nc) as tc, ExitStack() as ctx:
        const = ctx.enter_context(tc.tile_pool(name="const", bufs=1))
        widep = ctx.enter_context(tc.tile_pool(name="widep", bufs=3))
        work = ctx.enter_context(tc.tile_pool(name="work", bufs=3))
        outp = ctx.enter_context(tc.tile_pool(name="outp", bufs=3))
        psum = ctx.enter_context(tc.tile_pool(name="psum", bufs=2, space="PSUM"))

        # ---- load constants ----
        xT = const.tile([IN_F, SHP], F32)
        nc.sync.dma_start(xT[:], xT_in[:])
        nbr = const.tile([128, max(sumd, 1)], I32)
        nc.sync.dma_start(nbr[:], nbr_in[:])
        dinv = const.tile([128, NBLK], F32)
        nc.sync.dma_start(dinv[:], dinv_in[:])
        wt = {}
        for name, shp in [("W1", [IN_F, H]), ("W2", [H, H]), ("W3", [H, H]),
                          ("Wl", [H, C_OUT]), ("B1", [128, H]), ("B2", [128, H]),
                          ("B3", [128, H]), ("BL", [128, C_OUT])]:
            t = const.tile(shp, F32, tag=name)
            nc.sync.dma_start(t[:], w_in[name][:])
            wt[name] = t
        ident = const.tile([128, 128], F32)
        make_identity(nc, ident[:])
        zt = const.tile([128, H], F32)
        nc.vector.memset(zt[:], 0.0)

        # ---- layer-1 table: u1 = dinv * (x @ W1), per block ----
        for b in range(NBLK):
            vP = psum.tile([128, H], F32, tag="vP")
            nc.tensor.matmul(vP[:], lhsT=xT[:, b * 128:(b + 1) * 128],
                             rhs=wt["W1"][:], start=True, stop=True)
            ub = work.tile([128, H], F32, tag="ub")
            nc.vector.tensor_scalar(ub[:], vP[:], dinv[:, b:b + 1], None,
                                    op0=mybir.AluOpType.mult)
            nc.sync.dma_start(u_shard[0][b * 128:(b + 1) * 128, :], ub[:])

        for l in range(3):
            nc.gpsimd.collective_compute(
                "AllGather", mybir.AluOpType.bypass,
                replica_groups=[list(range(CORES))],
                ins=[u_shard[l][:]], outs=[u_table[l][:]],
            )
            for b in range(NBLK):
                d = D[b]
                wide = widep.tile([128, d + 1, H], F32, tag="wide")
                # self-loop column from own shard (contiguous rows)
                nc.sync.dma_start(wide[:, 0, :],
                                  u_shard[l][b * 128:(b + 1) * 128, :])
                for j in range(d):
                    col = offs[b] + j
                    nc.gpsimd.indirect_dma_start(
                        out=wide[:, 1 + j, :], out_offset=None,
                        in_=u_table[l][:],
                        in_offset=bass.IndirectOffsetOnAxis(
                            ap=nbr[:, col:col + 1], axis=0))
                s = work.tile([128, H], F32, tag="s")
                nc.vector.tensor_reduce(
                    s[:], wide[:].rearrange("p j d -> p d j"),
                    axis=mybir.AxisListType.X, op=mybir.AluOpType.add)
                # h = leaky(dinv*s + b)
                t1 = work.tile([128, H], F32, tag="t1")
                nc.vector.tensor_scalar(t1[:], s[:], dinv[:, b:b + 1], None,
                                        op0=mybir.AluOpType.mult)
                t2 = work.tile([128, H], F32, tag="t2")
                nc.vector.tensor_tensor(t2[:], t1[:], wt[B_of[l]][:],
                                        op=mybir.AluOpType.add)
                t3 = work.tile([128, H], F32, tag="t3")
                nc.vector.tensor_scalar(t3[:], t2[:], SLOPE, None,
                                        op0=mybir.AluOpType.mult)
                h = work.tile([128, H], F32, tag="h")
                nc.vector.tensor_tensor(h[:], t2[:], t3[:],
                                        op=mybir.AluOpType.max)
                if l < 2:
                    hs = work.tile([128, H], F32, tag="hs")
                    nc.vector.tensor_scalar(hs[:], h[:], dinv[:, b:b + 1], None,
                                            op0=mybir.AluOpType.mult)
                    trP = psum.tile([H, 128], F32, tag="trP")
                    nc.tensor.transpose(trP[:], hs[:], ident[:])
                    hsT = work.tile([H, 128], F32, tag="hsT")
                    nc.vector.tensor_copy(hsT[:], trP[:])
                    vP = psum.tile([128, H], F32, tag="vP")
                    nc.tensor.matmul(vP[:], lhsT=hsT[:], rhs=wt[W_next[l]][:],
                                     start=True, stop=True)
                    ub = work.tile([128, H], F32, tag="ub")
                    nc.vector.tensor_copy(ub[:], vP[:])
                    if b == NBLK - 1 and SH < SHP:
                        nreal = SH - (NBLK - 1) * 128
                        nc.sync.dma_start(
                            u_shard[l + 1][b * 128:b * 128 + nreal, :],
                            ub[:nreal, :])
                        nc.sync.dma_start(
                            u_shard[l + 1][SH:SHP, :], zt[:SHP - SH, :])
                    else:
                        nc.sync.dma_start(
                            u_shard[l + 1][b * 128:(b + 1) * 128, :], ub[:])
                else:
                    trP = psum.tile([H, 128], F32, tag="trP")
                    nc.tensor.transpose(trP[:], h[:], ident[:])
                    hT = work.tile([H, 128], F32, tag="hsT")
                    nc.vector.tensor_copy(hT[:], trP[:])
                    oP = psum.tile([128, C_OUT], F32, tag="oP")
                    nc.tensor.matmul(oP[:], lhsT=hT[:], rhs=wt["Wl"][:],
                                     start=True, stop=True)
                    o = outp.tile([128, C_OUT], F32, tag="o")
                    nc.vector.tensor_tensor(o[:], oP[:], wt["BL"][:],
                                            op=mybir.AluOpType.add)
                    nc.sync.dma_start(out_dram[b * 128:(b + 1) * 128, :], o[:])
    nc.compile()
    return nc


def _ensure_ntff_hook():
    """The agent image's antenv lacks axon_hooks; shim it and register the
    ctypes NTFF profiling hook so trace=True works under axon."""
    import sys as _sys
    import types
    try:
        import antenv.axon_hooks  # noqa: F401
        return
    except ImportError:
        pass
    mod = types.ModuleType("antenv.axon_hooks")
    _h = [None]
    mod.set_axon_ntff_profile_hook = lambda hook: _h.__setitem__(0, hook)
    mod.get_axon_ntff_profile_hook = lambda: _h[0]
    _sys.modules["antenv.axon_hooks"] = mod
    try:
        from trn_agent_boot.trn_boot import _ntff_profile_via_ctypes
        hook = _ntff_profile_via_ctypes("/opt/axon/libaxon_pjrt.so")
        if hook is not None:
            mod.set_axon_ntff_profile_hook(hook)
    except Exception:
        pass


def kernel(x, edge_index, W1, b1, W2, b2, W3, b3, Wl, bl):
    global LAST_RESULTS
    x = np.asarray(x, dtype=np.float32)
    xTs, nbrs, dinv_blks, D, offs, poss = _host_prep(x, edge_index)
    sumd = offs[-1]

    nc = _build_bass(D, offs, sumd)

    shared = {
        "W1": np.asarray(W1, np.float32), "W2": np.asarray(W2, np.float32),
        "W3": np.asarray(W3, np.float32), "Wl": np.asarray(Wl, np.float32),
        "B1": np.tile(np.asarray(b1, np.float32), (128, 1)),
        "B2": np.tile(np.asarray(b2, np.float32), (128, 1)),
        "B3": np.tile(np.asarray(b3, np.float32), (128, 1)),
        "BL": np.tile(np.asarray(bl, np.float32), (128, 1)),
    }
    in_maps = []
    for c in range(CORES):
        m = dict(shared)
        m["xT"] = xTs[c]
        m["nbr"] = nbrs[c] if sumd else np.zeros((128, 1), np.int32)
        m["dinv_blk"] = dinv_blks[c]
        in_maps.append(m)

    trace = bool(int(os.environ.get("GCN_TRACE", "0")))
    if trace:
        _ensure_ntff_hook()
    res = run_bass_kernel_spmd(nc, in_maps, list(range(CORES)), trace=trace)
    LAST_RESULTS = res

    out = np.empty((N, C_OUT), dtype=np.float32)
    for c in range(CORES):
        shard = res.results[c]["out_s"]
        out[c * SH:(c + 1) * SH] = shard[poss[c]]
    return out
